# revision 14
# baseline (speedup 1.0000x reference)
"""Trainium2 Bass kernel for nn_DET_PROB (hierarchical segmented cumprod).

Reference semantics (per row):
  c0 = cumprod(dc0)                       [B, 8]
  c1 = cumprod(dc1 grouped by 16)         [B, 8, 16]
  c2 = cumprod(dc2 grouped by 16)         [B, 128, 16]
  out[g=(a0,a1), k] = c0[a0] * c1[a0,a1] * c2[g, k]

Strategy: pure data parallel over 8 NeuronCores (batch split). Per core:
- dc0/dc1 (2.1 MiB) are preloaded whole and levels 0+1 are computed ONCE
  upfront with the hardware prefix-scan (state = data0*state + data1;
  zeroing data0 at segment starts and seeding data1 with the prefix-folded
  first element makes one scan do every segment). The transient buffers
  live in a scoped pool that is freed before the main loop.
- The per-tile loop is just: dc2 load (ACT HWDGE queue - measured 396
  GB/s vs SP's 258) -> level-2 segmented cumprod -> store (Pool SWDGE
  queue). The output rides HBM as bf16 (host widens back to f32): the
  scan's f32 state is rounded once per element, 3.9e-3 max rel err vs the
  2e-2 gate, and it halves store traffic. bf16 *inputs* would compound
  ~16 rounded factors per output (3.3e-2 on seed-0 data - over the gate),
  so reads stay f32.
- Level-2 (MODE="scan") is one contiguous tensor_tensor_scan per tile
  writing the bf16 output tile directly (no separate downcast op); the
  prefix is folded into each group's seed via the zero-companion trick.
  Ping-pong persistent companions are zeroed once - only group-start
  slots are rewritten per tile. The strided 16-step mul ladder
  (MODE="ladder_pair", pair-interleaved) measures within 2% of the scan;
  both are DVE-rate-bound (~0.5-0.6 elem/cycle for f32).
"""
import os as _os
import numpy as np
import concourse.bacc as bacc
import concourse.tile as tile
import concourse.mybir as mybir
from concourse.bass_utils import run_bass_kernel_spmd
from contextlib import ExitStack

F32 = mybir.dt.float32
BF16 = mybir.dt.bfloat16
P = 128
B0, B1, B2 = 8, 16, 16
BATCH = 32768
N_CORES = 8
ROWS_PER_CORE = BATCH // N_CORES  # 4096
R = 4  # rows per partition per tile
T = ROWS_PER_CORE // (P * R)  # 8 tiles

MODE = _os.environ.get("K_MODE", "scan")
# load queue per tile: "sp" all SP; "alt" even/odd tiles SP/ACT;
# "pairalt" whole pairs alternate SP/ACT
LOADQ = _os.environ.get("K_LOADQ", "act")
# store queue: "act" | "pool" (SWDGE) | "alt" (even SP, odd ACT) |
# "oppalt" (queue opposite the pair's load queue)
STOREQ = _os.environ.get("K_STOREQ", "pool")
OB2 = int(_os.environ.get("K_OB2", "2"))
SCRATCH = int(_os.environ.get("K_SCRATCH", "16384"))


def _build(n_rows: int, num_devices, loop_n=None, plan=None):
    """loop_n: if set, wrap the per-tile loop in a hardware For_i that
    repeats it loop_n times (benchmark-only; each repetition recomputes the
    same result). The dc0/dc1 preload + level-0/1 prefix precompute stay
    outside the loop, mirroring how the persistent scan operands were set
    up outside it in earlier revisions."""
    assert n_rows == ROWS_PER_CORE
    nT = n_rows // (P * R)
    F2 = R * B0 * B1 * B2  # 8192 per-partition elems per dc2 tile
    X0 = nT * R * B0       # 256  (t, r, c) dc0 elems per partition
    X1 = nT * R * B0 * B1  # 4096 (t, r, g, k) dc1 elems per partition

    nc = bacc.Bacc("TRN2", debug=False, num_devices=num_devices,
                   dynamic_dma_scratch_size=SCRATCH)
    dc0 = nc.dram_tensor("dc0", [n_rows, B0], F32, kind="ExternalInput").ap()
    dc1 = nc.dram_tensor("dc1", [n_rows, B0 * B1], F32, kind="ExternalInput").ap()
    dc2 = nc.dram_tensor("dc2", [n_rows, B0 * B1 * B2], F32, kind="ExternalInput").ap()
    out = nc.dram_tensor("out", [n_rows, B0 * B1 * B2], BF16, kind="ExternalOutput").ap()

    mult = mybir.AluOpType.mult
    add = mybir.AluOpType.add

    def rows_view(ap, row0, c):
        # partition p holds R consecutive rows starting at row0 + p*R
        return ap[row0 : row0 + P * R, :].rearrange("(p r) c -> p r c", r=R)

    with tile.TileContext(nc) as tc, ExitStack() as ctx:
        persist = ctx.enter_context(tc.tile_pool(name="persist", bufs=1))
        prefix = persist.tile([P, X1], F32)  # c0*c1 for every (row, group)

        # ---- one-shot: compute the level-0/1 prefix for the whole core,
        # tile by tile (same op/AP shapes as the proven per-tile kernel),
        # into the persistent `prefix` buffer ----
        F0, F1 = R * B0, R * B0 * B1
        with tc.tile_pool(name="pre", bufs=2) as pre, tc.tile_pool(
            name="prez", bufs=1
        ) as prez:
            z0 = prez.tile([P, F0], F32)
            z1 = prez.tile([P, F1], F32)
            nc.vector.memset(z0[:], 0.0)
            nc.vector.memset(z1[:], 0.0)
            for t in range(nT):
                row0 = t * P * R
                s0t = pre.tile([P, F0], F32)
                s1t = pre.tile([P, F1], F32)
                c0t = pre.tile([P, F0], F32)
                s0, s1 = s0t[:], s1t[:]
                nc.sync.dma_start(
                    out=s0.rearrange("p (r c) -> p r c", c=B0),
                    in_=rows_view(dc0, row0, B0),
                )
                nc.sync.dma_start(
                    out=s1.rearrange("p (r c) -> p r c", c=B0 * B1),
                    in_=rows_view(dc1, row0, B0 * B1),
                )
                # level 0: segmented cumprod over rows of 8
                b0 = s0.rearrange("p (x c) -> p x c", c=B0)[:, :, 0:1]
                z0b = z0[:].rearrange("p (x c) -> p x c", c=B0)[:, :, 0:1]
                nc.vector.tensor_scalar_mul(z0b, b0, 1.0)
                nc.vector.memset(b0, 0.0)
                nc.vector.tensor_tensor_scan(c0t[:], s0, z0[:], 0.0, mult, add)
                # level 1: fold c0 into group starts, segmented cumprod of 16
                b1 = s1.rearrange("p (x k) -> p x k", k=B1)[:, :, 0:1]
                z1b = z1[:].rearrange("p (x k) -> p x k", k=B1)[:, :, 0:1]
                c0u = c0t[:].rearrange("p (x c) -> p x c", c=1)
                nc.vector.tensor_mul(z1b, b1, c0u)
                nc.vector.memset(b1, 0.0)
                nc.vector.tensor_tensor_scan(
                    prefix[:, t * F1 : (t + 1) * F1], s1, z1[:], 0.0, mult, add
                )

        io2 = ctx.enter_context(
            tc.tile_pool(name="io2", bufs=2 if MODE == "scan" else 4)
        )
        ob2 = ctx.enter_context(tc.tile_pool(name="ob2", bufs=OB2))
        z2s = None
        if MODE == "scan":
            # ping-pong scan companions: zero once; only group-start slots
            # are rewritten per tile, the rest stays zero forever
            z2a = persist.tile([P, F2], F32)
            z2b_t = persist.tile([P, F2], F32)
            nc.vector.memset(z2a[:], 0.0)
            nc.vector.memset(z2b_t[:], 0.0)
            z2s = [z2a, z2b_t]

        if loop_n is not None:
            ctx.enter_context(tc.For_i(0, loop_n, 1))

        def ladder_ops(s2, t):
            """Thunk per level-2 ladder step for tile t (prefix folded into
            element 0, then 15 dependent in-place strided muls)."""
            g2 = s2.rearrange("p (g c) -> p g c", c=B2)
            pu = prefix[:, t * R * B0 * B1 : (t + 1) * R * B0 * B1].rearrange(
                "p (g c) -> p g c", c=1
            )
            yield lambda: nc.vector.tensor_mul(g2[:, :, 0:1], g2[:, :, 0:1], pu)
            for k in range(1, B2):
                yield (
                    lambda k=k: nc.vector.tensor_mul(
                        g2[:, :, k : k + 1], g2[:, :, k : k + 1], g2[:, :, k - 1 : k]
                    )
                )

        def emit_store(s2, row0, t=0):
            o2 = ob2.tile([P, F2], BF16)
            nc.scalar.copy(o2[:], s2)
            if STOREQ == "pool":
                eng = nc.gpsimd
            elif STOREQ == "alt":
                eng = nc.sync if t % 2 == 0 else nc.scalar
            elif STOREQ == "oppalt":
                eng = nc.scalar if (t // 2) % 2 == 0 else nc.sync
            else:
                eng = nc.scalar
            eng.dma_start(
                out=rows_view(out, row0, B0 * B1 * B2),
                in_=o2[:].rearrange("p (r c) -> p r c", c=B0 * B1 * B2),
            )

        def emit_load(t):
            t2 = io2.tile([P, F2], F32)
            s2 = t2[:]
            if LOADQ == "alt":
                eng = nc.scalar if t % 2 == 1 else nc.sync
            elif LOADQ == "pairalt":
                eng = nc.scalar if (t // 2) % 2 == 1 else nc.sync
            elif LOADQ == "act":
                eng = nc.scalar
            else:
                eng = nc.sync
            eng.dma_start(
                out=s2.rearrange("p (r c) -> p r c", c=B0 * B1 * B2),
                in_=rows_view(dc2, t * P * R, B0 * B1 * B2),
            )
            return s2

        if MODE == "scan":
            # one contiguous segmented scan per tile, writing bf16 output
            # directly (scan state stays f32; each output rounded once)
            for t in range(nT):
                s2 = emit_load(t)
                z2 = z2s[t % 2]
                s2b = s2.rearrange("p (g c) -> p g c", c=B2)[:, :, 0:1]
                z2b = z2[:].rearrange("p (g c) -> p g c", c=B2)[:, :, 0:1]
                pu = prefix[:, t * F1 : (t + 1) * F1].rearrange("p (g c) -> p g c", c=1)
                nc.vector.tensor_mul(z2b, s2b, pu)
                nc.vector.memset(s2b, 0.0)
                o2 = ob2.tile([P, F2], BF16)
                nc.vector.tensor_tensor_scan(o2[:], s2, z2[:], 0.0, mult, add)
                eng = nc.gpsimd if STOREQ == "pool" else nc.scalar
                eng.dma_start(
                    out=rows_view(out, t * P * R, B0 * B1 * B2),
                    in_=o2[:].rearrange("p (r c) -> p r c", c=B0 * B1 * B2),
                )
        elif MODE == "ladder_pair":
            assert nT % 2 == 0
            for t in range(0, nT, 2):
                sa = emit_load(t)
                sb = emit_load(t + 1)
                for opa, opb in zip(ladder_ops(sa, t), ladder_ops(sb, t + 1)):
                    opa()
                    opb()
                emit_store(sa, t * P * R, t)
                emit_store(sb, (t + 1) * P * R, t + 1)
        elif MODE == "ladder":
            for t in range(nT):
                s2 = emit_load(t)
                for op in ladder_ops(s2, t):
                    op()
                emit_store(s2, t * P * R)
        else:
            raise ValueError(MODE)
    nc.compile()
    return nc


_CACHED = None


def _get_program():
    global _CACHED
    if _CACHED is None:
        _CACHED = _build(ROWS_PER_CORE, N_CORES)
    return _CACHED


def run(inputs, trace=False, **kwargs):
    """Shard inputs over 8 cores, run SPMD, gather. Returns (out, BassKernelResults)."""
    dc0 = np.ascontiguousarray(inputs["dc0"], dtype=np.float32)
    dc1 = np.ascontiguousarray(inputs["dc1"], dtype=np.float32)
    dc2 = np.ascontiguousarray(inputs["dc2"], dtype=np.float32)
    assert dc0.shape == (BATCH, B0) and dc1.shape == (BATCH, B0 * B1)
    assert dc2.shape == (BATCH, B0 * B1 * B2)

    nc = _get_program()
    in_maps = []
    for c in range(N_CORES):
        sl = slice(c * ROWS_PER_CORE, (c + 1) * ROWS_PER_CORE)
        in_maps.append({"dc0": dc0[sl], "dc1": dc1[sl], "dc2": dc2[sl]})
    res = run_bass_kernel_spmd(
        nc, in_maps, core_ids=list(range(N_CORES)), trace=trace, **kwargs
    )
    out = np.concatenate([res.results[c]["out"] for c in range(N_CORES)], axis=0)
    return out.astype(np.float32), res


def kernel(**inputs) -> np.ndarray:
    out, _ = run(inputs, trace=False)
    return out


# revision 15
# speedup vs baseline: 4.5575x; 4.5575x over previous
"""Trainium2 Bass kernel for nn_DET_PROB (hierarchical segmented cumprod).

Reference semantics (per row):
  c0 = cumprod(dc0)                       [B, 8]
  c1 = cumprod(dc1 grouped by 16)         [B, 8, 16]
  c2 = cumprod(dc2 grouped by 16)         [B, 128, 16]
  out[g=(a0,a1), k] = c0[a0] * c1[a0,a1] * c2[g, k]

Strategy: pure data parallel over 8 NeuronCores (batch split). Per core:
- dc0/dc1 (2.1 MiB) are preloaded whole and levels 0+1 are computed ONCE
  upfront with the hardware prefix-scan (state = data0*state + data1;
  zeroing data0 at segment starts and seeding data1 with the prefix-folded
  first element makes one scan do every segment). The transient buffers
  live in a scoped pool that is freed before the main loop.
- The per-tile loop is just: dc2 load (SP queue) -> level-2 cumprod on DVE
  -> f32->bf16 downcast on the otherwise-idle ACT engine -> store (ACT
  queue). The output rides HBM as bf16 (host widens back to f32): max rel
  err from that one rounding is 3.9e-3 vs the 2e-2 gate, and it halves
  store traffic. bf16 *inputs* would compound ~16 rounded factors per
  output (3.3e-2 on the seed-0 data - over the gate), so reads stay f32.
- Level-2 uses 16 in-place strided tensor_mul ops per tile (a dependent
  ladder down each group of 16), with the level-0/1 prefix folded into
  element 0. Two tiles' ladders are interleaved instruction-by-instruction
  so the DVE never stalls on its own pipeline latency between dependent
  ops (measured: the dependent-op turnaround, not DMA, was the bottleneck
  of the non-interleaved kernel).
"""
import os as _os
import numpy as np
import concourse.bacc as bacc
import concourse.tile as tile
import concourse.mybir as mybir
from concourse.bass_utils import run_bass_kernel_spmd
from contextlib import ExitStack

F32 = mybir.dt.float32
BF16 = mybir.dt.bfloat16
P = 128
B0, B1, B2 = 8, 16, 16
BATCH = 32768
N_CORES = 8
ROWS_PER_CORE = BATCH // N_CORES  # 4096
R = 4  # rows per partition per tile
T = ROWS_PER_CORE // (P * R)  # 8 tiles

MODE = _os.environ.get("K_MODE", "ladder_pair")
# load queue per tile: "sp" all SP; "alt" even/odd tiles SP/ACT;
# "pairalt" whole pairs alternate SP/ACT
LOADQ = _os.environ.get("K_LOADQ", "sp")
# store queue: "act" | "pool" (SWDGE) | "alt" (even SP, odd ACT) |
# "oppalt" (queue opposite the pair's load queue)
STOREQ = _os.environ.get("K_STOREQ", "pool")
OB2 = int(_os.environ.get("K_OB2", "2"))
SCRATCH = int(_os.environ.get("K_SCRATCH", "16384"))
# engine for the per-tile scan-seed fold ops: "dve" | "pool" (GPSIMD)
FOLD = _os.environ.get("K_FOLD", "dve")
# free-dim elems per tile scanned on GPSIMD instead of DVE (0 = off;
# must be a multiple of 16 so the split lands on group boundaries)
SCANSPLIT = int(_os.environ.get("K_SCANSPLIT", "0"))
# f32->bf16 downcast engine (ladder modes): "act" | "pool" | "split"
# (split = even tiles on ACT, odd on GPSIMD - keeps the ACT sequencer
# free so load triggers aren't serialized behind 6.8us copy ops)
CONV = _os.environ.get("K_CONV", "act")


def _build(n_rows: int, num_devices, loop_n=None, plan=None):
    """loop_n: if set, wrap the per-tile loop in a hardware For_i that
    repeats it loop_n times (benchmark-only; each repetition recomputes the
    same result). The dc0/dc1 preload + level-0/1 prefix precompute stay
    outside the loop, mirroring how the persistent scan operands were set
    up outside it in earlier revisions."""
    assert n_rows == ROWS_PER_CORE
    nT = n_rows // (P * R)
    F2 = R * B0 * B1 * B2  # 8192 per-partition elems per dc2 tile
    X0 = nT * R * B0       # 256  (t, r, c) dc0 elems per partition
    X1 = nT * R * B0 * B1  # 4096 (t, r, g, k) dc1 elems per partition

    nc = bacc.Bacc("TRN2", debug=False, num_devices=num_devices,
                   dynamic_dma_scratch_size=SCRATCH)
    dc0 = nc.dram_tensor("dc0", [n_rows, B0], F32, kind="ExternalInput").ap()
    dc1 = nc.dram_tensor("dc1", [n_rows, B0 * B1], F32, kind="ExternalInput").ap()
    dc2 = nc.dram_tensor("dc2", [n_rows, B0 * B1 * B2], F32, kind="ExternalInput").ap()
    out = nc.dram_tensor("out", [n_rows, B0 * B1 * B2], BF16, kind="ExternalOutput").ap()

    mult = mybir.AluOpType.mult
    add = mybir.AluOpType.add

    def rows_view(ap, row0, c):
        # partition p holds R consecutive rows starting at row0 + p*R
        return ap[row0 : row0 + P * R, :].rearrange("(p r) c -> p r c", r=R)

    with tile.TileContext(nc) as tc, ExitStack() as ctx:
        persist = ctx.enter_context(tc.tile_pool(name="persist", bufs=1))
        prefix = persist.tile([P, X1], F32)  # c0*c1 for every (row, group)

        # ---- one-shot: compute the level-0/1 prefix for the whole core,
        # tile by tile (same op/AP shapes as the proven per-tile kernel),
        # into the persistent `prefix` buffer ----
        F0, F1 = R * B0, R * B0 * B1
        with tc.tile_pool(name="pre", bufs=2) as pre, tc.tile_pool(
            name="prez", bufs=1
        ) as prez:
            z0 = prez.tile([P, F0], F32)
            z1 = prez.tile([P, F1], F32)
            nc.vector.memset(z0[:], 0.0)
            nc.vector.memset(z1[:], 0.0)
            for t in range(nT):
                row0 = t * P * R
                s0t = pre.tile([P, F0], F32)
                s1t = pre.tile([P, F1], F32)
                c0t = pre.tile([P, F0], F32)
                s0, s1 = s0t[:], s1t[:]
                nc.sync.dma_start(
                    out=s0.rearrange("p (r c) -> p r c", c=B0),
                    in_=rows_view(dc0, row0, B0),
                )
                nc.sync.dma_start(
                    out=s1.rearrange("p (r c) -> p r c", c=B0 * B1),
                    in_=rows_view(dc1, row0, B0 * B1),
                )
                # level 0: segmented cumprod over rows of 8
                b0 = s0.rearrange("p (x c) -> p x c", c=B0)[:, :, 0:1]
                z0b = z0[:].rearrange("p (x c) -> p x c", c=B0)[:, :, 0:1]
                nc.vector.tensor_scalar_mul(z0b, b0, 1.0)
                nc.vector.memset(b0, 0.0)
                nc.vector.tensor_tensor_scan(c0t[:], s0, z0[:], 0.0, mult, add)
                # level 1: fold c0 into group starts, segmented cumprod of 16
                b1 = s1.rearrange("p (x k) -> p x k", k=B1)[:, :, 0:1]
                z1b = z1[:].rearrange("p (x k) -> p x k", k=B1)[:, :, 0:1]
                c0u = c0t[:].rearrange("p (x c) -> p x c", c=1)
                nc.vector.tensor_mul(z1b, b1, c0u)
                nc.vector.memset(b1, 0.0)
                nc.vector.tensor_tensor_scan(
                    prefix[:, t * F1 : (t + 1) * F1], s1, z1[:], 0.0, mult, add
                )

        io2 = ctx.enter_context(
            tc.tile_pool(name="io2", bufs=2 if MODE == "scan" else 4)
        )
        ob2 = ctx.enter_context(tc.tile_pool(name="ob2", bufs=OB2))
        z2s = None
        if MODE == "scan":
            # ping-pong scan companions: zero once; only group-start slots
            # are rewritten per tile, the rest stays zero forever
            z2a = persist.tile([P, F2], F32)
            z2b_t = persist.tile([P, F2], F32)
            nc.vector.memset(z2a[:], 0.0)
            nc.vector.memset(z2b_t[:], 0.0)
            z2s = [z2a, z2b_t]

        if loop_n is not None:
            ctx.enter_context(tc.For_i(0, loop_n, 1))

        def ladder_ops(s2, t):
            """Thunk per level-2 ladder step for tile t (prefix folded into
            element 0, then 15 dependent in-place strided muls)."""
            g2 = s2.rearrange("p (g c) -> p g c", c=B2)
            pu = prefix[:, t * R * B0 * B1 : (t + 1) * R * B0 * B1].rearrange(
                "p (g c) -> p g c", c=1
            )
            yield lambda: nc.vector.tensor_mul(g2[:, :, 0:1], g2[:, :, 0:1], pu)
            for k in range(1, B2):
                yield (
                    lambda k=k: nc.vector.tensor_mul(
                        g2[:, :, k : k + 1], g2[:, :, k : k + 1], g2[:, :, k - 1 : k]
                    )
                )

        def emit_store(s2, row0, t=0):
            o2 = ob2.tile([P, F2], BF16)
            if CONV == "pool" or (CONV == "split" and t % 2 == 1):
                nc.gpsimd.tensor_scalar_mul(o2[:], s2, 1.0)
            else:
                nc.scalar.copy(o2[:], s2)
            if STOREQ == "pool":
                eng = nc.gpsimd
            elif STOREQ == "alt":
                eng = nc.sync if t % 2 == 0 else nc.scalar
            elif STOREQ == "oppalt":
                eng = nc.scalar if (t // 2) % 2 == 0 else nc.sync
            else:
                eng = nc.scalar
            eng.dma_start(
                out=rows_view(out, row0, B0 * B1 * B2),
                in_=o2[:].rearrange("p (r c) -> p r c", c=B0 * B1 * B2),
            )

        def emit_load(t):
            t2 = io2.tile([P, F2], F32)
            s2 = t2[:]
            if LOADQ == "alt":
                eng = nc.scalar if t % 2 == 1 else nc.sync
            elif LOADQ == "pairalt":
                eng = nc.scalar if (t // 2) % 2 == 1 else nc.sync
            elif LOADQ == "act":
                eng = nc.scalar
            else:
                eng = nc.sync
            eng.dma_start(
                out=s2.rearrange("p (r c) -> p r c", c=B0 * B1 * B2),
                in_=rows_view(dc2, t * P * R, B0 * B1 * B2),
            )
            return s2

        if MODE == "scan":
            # one contiguous segmented scan per tile, writing bf16 output
            # directly (scan state stays f32; each output rounded once)
            for t in range(nT):
                s2 = emit_load(t)
                z2 = z2s[t % 2]
                s2b = s2.rearrange("p (g c) -> p g c", c=B2)[:, :, 0:1]
                z2b = z2[:].rearrange("p (g c) -> p g c", c=B2)[:, :, 0:1]
                pu = prefix[:, t * F1 : (t + 1) * F1].rearrange("p (g c) -> p g c", c=1)
                feng = nc.gpsimd if FOLD == "pool" else nc.vector
                feng.tensor_mul(z2b, s2b, pu)
                feng.memset(s2b, 0.0)
                o2 = ob2.tile([P, F2], BF16)
                if SCANSPLIT:
                    fs = F2 - SCANSPLIT
                    nc.vector.tensor_tensor_scan(o2[:, :fs], s2[:, :fs], z2[:, :fs], 0.0, mult, add)
                    nc.gpsimd.tensor_tensor_scan(o2[:, fs:], s2[:, fs:], z2[:, fs:], 0.0, mult, add)
                else:
                    nc.vector.tensor_tensor_scan(o2[:], s2, z2[:], 0.0, mult, add)
                eng = nc.gpsimd if STOREQ == "pool" else nc.scalar
                eng.dma_start(
                    out=rows_view(out, t * P * R, B0 * B1 * B2),
                    in_=o2[:].rearrange("p (r c) -> p r c", c=B0 * B1 * B2),
                )
        elif MODE == "ladder_pair":
            assert nT % 2 == 0
            for t in range(0, nT, 2):
                sa = emit_load(t)
                sb = emit_load(t + 1)
                for opa, opb in zip(ladder_ops(sa, t), ladder_ops(sb, t + 1)):
                    opa()
                    opb()
                emit_store(sa, t * P * R, t)
                emit_store(sb, (t + 1) * P * R, t + 1)
        elif MODE == "ladder":
            for t in range(nT):
                s2 = emit_load(t)
                for op in ladder_ops(s2, t):
                    op()
                emit_store(s2, t * P * R)
        else:
            raise ValueError(MODE)
    nc.compile()
    return nc


_CACHED = None


def _get_program():
    global _CACHED
    if _CACHED is None:
        _CACHED = _build(ROWS_PER_CORE, N_CORES)
    return _CACHED


def run(inputs, trace=False, **kwargs):
    """Shard inputs over 8 cores, run SPMD, gather. Returns (out, BassKernelResults)."""
    dc0 = np.ascontiguousarray(inputs["dc0"], dtype=np.float32)
    dc1 = np.ascontiguousarray(inputs["dc1"], dtype=np.float32)
    dc2 = np.ascontiguousarray(inputs["dc2"], dtype=np.float32)
    assert dc0.shape == (BATCH, B0) and dc1.shape == (BATCH, B0 * B1)
    assert dc2.shape == (BATCH, B0 * B1 * B2)

    nc = _get_program()
    in_maps = []
    for c in range(N_CORES):
        sl = slice(c * ROWS_PER_CORE, (c + 1) * ROWS_PER_CORE)
        in_maps.append({"dc0": dc0[sl], "dc1": dc1[sl], "dc2": dc2[sl]})
    res = run_bass_kernel_spmd(
        nc, in_maps, core_ids=list(range(N_CORES)), trace=trace, **kwargs
    )
    out = np.concatenate([res.results[c]["out"] for c in range(N_CORES)], axis=0)
    return out.astype(np.float32), res


def kernel(**inputs) -> np.ndarray:
    out, _ = run(inputs, trace=False)
    return out


# revision 16
# speedup vs baseline: 4.6115x; 1.0119x over previous
"""Trainium2 Bass kernel for nn_DET_PROB (hierarchical segmented cumprod).

Reference semantics (per row):
  c0 = cumprod(dc0)                       [B, 8]
  c1 = cumprod(dc1 grouped by 16)         [B, 8, 16]
  c2 = cumprod(dc2 grouped by 16)         [B, 128, 16]
  out[g=(a0,a1), k] = c0[a0] * c1[a0,a1] * c2[g, k]

Strategy: pure data parallel over 8 NeuronCores (batch split). Per core:
- dc0/dc1 (2.1 MiB) are preloaded whole and levels 0+1 are computed ONCE
  upfront with the hardware prefix-scan (state = data0*state + data1;
  zeroing data0 at segment starts and seeding data1 with the prefix-folded
  first element makes one scan do every segment). The transient buffers
  live in a scoped pool that is freed before the main loop.
- The per-tile loop is just: dc2 load (SP queue) -> level-2 cumprod on DVE
  -> f32->bf16 downcast on the otherwise-idle ACT engine -> store (Pool
  SWDGE queue). One job per sequencer - SP: load triggers, ACT engine:
  copies, Pool: store descriptors - measured faster than configs that
  couple a fast load queue with the copy engine's sequencer.
  The output rides HBM as bf16 (host widens back to f32): max rel
  err from that one rounding is 3.9e-3 vs the 2e-2 gate, and it halves
  store traffic. bf16 *inputs* would compound ~16 rounded factors per
  output (3.3e-2 on the seed-0 data - over the gate), so reads stay f32.
- Level-2 uses 16 in-place strided tensor_mul ops per tile (a dependent
  ladder down each group of 16), with the level-0/1 prefix folded into
  element 0. Two tiles' ladders are interleaved instruction-by-instruction
  so the DVE never stalls on its own pipeline latency between dependent
  ops (measured: the dependent-op turnaround, not DMA, was the bottleneck
  of the non-interleaved kernel).
"""
import os as _os
import numpy as np
import concourse.bacc as bacc
import concourse.tile as tile
import concourse.mybir as mybir
from concourse.bass_utils import run_bass_kernel_spmd
from contextlib import ExitStack

F32 = mybir.dt.float32
BF16 = mybir.dt.bfloat16
P = 128
B0, B1, B2 = 8, 16, 16
BATCH = 32768
N_CORES = 8
ROWS_PER_CORE = BATCH // N_CORES  # 4096
R = 4  # rows per partition per tile
T = ROWS_PER_CORE // (P * R)  # 8 tiles

MODE = _os.environ.get("K_MODE", "ladder_pair")
# load queue per tile: "sp" all SP; "alt" even/odd tiles SP/ACT;
# "pairalt" whole pairs alternate SP/ACT
LOADQ = _os.environ.get("K_LOADQ", "sp")
# store queue: "act" | "pool" (SWDGE) | "alt" (even SP, odd ACT) |
# "oppalt" (queue opposite the pair's load queue)
STOREQ = _os.environ.get("K_STOREQ", "pool")
OB2 = int(_os.environ.get("K_OB2", "2"))
SCRATCH = int(_os.environ.get("K_SCRATCH", "16384"))
# engine for the per-tile scan-seed fold ops: "dve" | "pool" (GPSIMD)
FOLD = _os.environ.get("K_FOLD", "dve")
# free-dim elems per tile scanned on GPSIMD instead of DVE (0 = off;
# must be a multiple of 16 so the split lands on group boundaries)
SCANSPLIT = int(_os.environ.get("K_SCANSPLIT", "0"))
# f32->bf16 downcast engine (ladder modes): "act" | "pool" | "split"
# (split = even tiles on ACT, odd on GPSIMD - keeps the ACT sequencer
# free so load triggers aren't serialized behind 6.8us copy ops)
CONV = _os.environ.get("K_CONV", "act")


def _build(n_rows: int, num_devices, loop_n=None, plan=None):
    """loop_n: if set, wrap the per-tile loop in a hardware For_i that
    repeats it loop_n times (benchmark-only; each repetition recomputes the
    same result). The dc0/dc1 preload + level-0/1 prefix precompute stay
    outside the loop, mirroring how the persistent scan operands were set
    up outside it in earlier revisions."""
    assert n_rows == ROWS_PER_CORE
    nT = n_rows // (P * R)
    F2 = R * B0 * B1 * B2  # 8192 per-partition elems per dc2 tile
    X0 = nT * R * B0       # 256  (t, r, c) dc0 elems per partition
    X1 = nT * R * B0 * B1  # 4096 (t, r, g, k) dc1 elems per partition

    nc = bacc.Bacc("TRN2", debug=False, num_devices=num_devices,
                   dynamic_dma_scratch_size=SCRATCH)
    dc0 = nc.dram_tensor("dc0", [n_rows, B0], F32, kind="ExternalInput").ap()
    dc1 = nc.dram_tensor("dc1", [n_rows, B0 * B1], F32, kind="ExternalInput").ap()
    dc2 = nc.dram_tensor("dc2", [n_rows, B0 * B1 * B2], F32, kind="ExternalInput").ap()
    out = nc.dram_tensor("out", [n_rows, B0 * B1 * B2], BF16, kind="ExternalOutput").ap()

    mult = mybir.AluOpType.mult
    add = mybir.AluOpType.add

    def rows_view(ap, row0, c):
        # partition p holds R consecutive rows starting at row0 + p*R
        return ap[row0 : row0 + P * R, :].rearrange("(p r) c -> p r c", r=R)

    with tile.TileContext(nc) as tc, ExitStack() as ctx:
        persist = ctx.enter_context(tc.tile_pool(name="persist", bufs=1))
        prefix = persist.tile([P, X1], F32)  # c0*c1 for every (row, group)

        # ---- one-shot: compute the level-0/1 prefix for the whole core,
        # tile by tile (same op/AP shapes as the proven per-tile kernel),
        # into the persistent `prefix` buffer ----
        F0, F1 = R * B0, R * B0 * B1
        with tc.tile_pool(name="pre", bufs=2) as pre, tc.tile_pool(
            name="prez", bufs=1
        ) as prez:
            z0 = prez.tile([P, F0], F32)
            z1 = prez.tile([P, F1], F32)
            nc.vector.memset(z0[:], 0.0)
            nc.vector.memset(z1[:], 0.0)
            for t in range(nT):
                row0 = t * P * R
                s0t = pre.tile([P, F0], F32)
                s1t = pre.tile([P, F1], F32)
                c0t = pre.tile([P, F0], F32)
                s0, s1 = s0t[:], s1t[:]
                nc.sync.dma_start(
                    out=s0.rearrange("p (r c) -> p r c", c=B0),
                    in_=rows_view(dc0, row0, B0),
                )
                nc.sync.dma_start(
                    out=s1.rearrange("p (r c) -> p r c", c=B0 * B1),
                    in_=rows_view(dc1, row0, B0 * B1),
                )
                # level 0: segmented cumprod over rows of 8
                b0 = s0.rearrange("p (x c) -> p x c", c=B0)[:, :, 0:1]
                z0b = z0[:].rearrange("p (x c) -> p x c", c=B0)[:, :, 0:1]
                nc.vector.tensor_scalar_mul(z0b, b0, 1.0)
                nc.vector.memset(b0, 0.0)
                nc.vector.tensor_tensor_scan(c0t[:], s0, z0[:], 0.0, mult, add)
                # level 1: fold c0 into group starts, segmented cumprod of 16
                b1 = s1.rearrange("p (x k) -> p x k", k=B1)[:, :, 0:1]
                z1b = z1[:].rearrange("p (x k) -> p x k", k=B1)[:, :, 0:1]
                c0u = c0t[:].rearrange("p (x c) -> p x c", c=1)
                nc.vector.tensor_mul(z1b, b1, c0u)
                nc.vector.memset(b1, 0.0)
                nc.vector.tensor_tensor_scan(
                    prefix[:, t * F1 : (t + 1) * F1], s1, z1[:], 0.0, mult, add
                )

        io2 = ctx.enter_context(
            tc.tile_pool(name="io2", bufs=2 if MODE == "scan" else 4)
        )
        ob2 = ctx.enter_context(tc.tile_pool(name="ob2", bufs=OB2))
        z2s = None
        if MODE == "scan":
            # ping-pong scan companions: zero once; only group-start slots
            # are rewritten per tile, the rest stays zero forever
            z2a = persist.tile([P, F2], F32)
            z2b_t = persist.tile([P, F2], F32)
            nc.vector.memset(z2a[:], 0.0)
            nc.vector.memset(z2b_t[:], 0.0)
            z2s = [z2a, z2b_t]

        if loop_n is not None:
            ctx.enter_context(tc.For_i(0, loop_n, 1))

        def ladder_ops(s2, t):
            """Thunk per level-2 ladder step for tile t (prefix folded into
            element 0, then 15 dependent in-place strided muls)."""
            g2 = s2.rearrange("p (g c) -> p g c", c=B2)
            pu = prefix[:, t * R * B0 * B1 : (t + 1) * R * B0 * B1].rearrange(
                "p (g c) -> p g c", c=1
            )
            yield lambda: nc.vector.tensor_mul(g2[:, :, 0:1], g2[:, :, 0:1], pu)
            for k in range(1, B2):
                yield (
                    lambda k=k: nc.vector.tensor_mul(
                        g2[:, :, k : k + 1], g2[:, :, k : k + 1], g2[:, :, k - 1 : k]
                    )
                )

        def emit_store(s2, row0, t=0):
            o2 = ob2.tile([P, F2], BF16)
            if CONV == "pool" or (CONV == "split" and t % 2 == 1):
                nc.gpsimd.tensor_scalar_mul(o2[:], s2, 1.0)
            else:
                nc.scalar.copy(o2[:], s2)
            if STOREQ == "pool":
                eng = nc.gpsimd
            elif STOREQ == "alt":
                eng = nc.sync if t % 2 == 0 else nc.scalar
            elif STOREQ == "oppalt":
                eng = nc.scalar if (t // 2) % 2 == 0 else nc.sync
            else:
                eng = nc.scalar
            eng.dma_start(
                out=rows_view(out, row0, B0 * B1 * B2),
                in_=o2[:].rearrange("p (r c) -> p r c", c=B0 * B1 * B2),
            )

        def emit_load(t):
            t2 = io2.tile([P, F2], F32)
            s2 = t2[:]
            if LOADQ == "alt":
                eng = nc.scalar if t % 2 == 1 else nc.sync
            elif LOADQ == "pairalt":
                eng = nc.scalar if (t // 2) % 2 == 1 else nc.sync
            elif LOADQ == "act":
                eng = nc.scalar
            else:
                eng = nc.sync
            eng.dma_start(
                out=s2.rearrange("p (r c) -> p r c", c=B0 * B1 * B2),
                in_=rows_view(dc2, t * P * R, B0 * B1 * B2),
            )
            return s2

        if MODE == "scan":
            # one contiguous segmented scan per tile, writing bf16 output
            # directly (scan state stays f32; each output rounded once)
            for t in range(nT):
                s2 = emit_load(t)
                z2 = z2s[t % 2]
                s2b = s2.rearrange("p (g c) -> p g c", c=B2)[:, :, 0:1]
                z2b = z2[:].rearrange("p (g c) -> p g c", c=B2)[:, :, 0:1]
                pu = prefix[:, t * F1 : (t + 1) * F1].rearrange("p (g c) -> p g c", c=1)
                feng = nc.gpsimd if FOLD == "pool" else nc.vector
                feng.tensor_mul(z2b, s2b, pu)
                feng.memset(s2b, 0.0)
                o2 = ob2.tile([P, F2], BF16)
                if SCANSPLIT:
                    fs = F2 - SCANSPLIT
                    nc.vector.tensor_tensor_scan(o2[:, :fs], s2[:, :fs], z2[:, :fs], 0.0, mult, add)
                    nc.gpsimd.tensor_tensor_scan(o2[:, fs:], s2[:, fs:], z2[:, fs:], 0.0, mult, add)
                else:
                    nc.vector.tensor_tensor_scan(o2[:], s2, z2[:], 0.0, mult, add)
                eng = nc.gpsimd if STOREQ == "pool" else nc.scalar
                eng.dma_start(
                    out=rows_view(out, t * P * R, B0 * B1 * B2),
                    in_=o2[:].rearrange("p (r c) -> p r c", c=B0 * B1 * B2),
                )
        elif MODE == "ladder_pair":
            assert nT % 2 == 0
            for t in range(0, nT, 2):
                sa = emit_load(t)
                sb = emit_load(t + 1)
                for opa, opb in zip(ladder_ops(sa, t), ladder_ops(sb, t + 1)):
                    opa()
                    opb()
                emit_store(sa, t * P * R, t)
                emit_store(sb, (t + 1) * P * R, t + 1)
        elif MODE == "ladder":
            for t in range(nT):
                s2 = emit_load(t)
                for op in ladder_ops(s2, t):
                    op()
                emit_store(s2, t * P * R)
        else:
            raise ValueError(MODE)
    nc.compile()
    return nc


_CACHED = None


def _get_program():
    global _CACHED
    if _CACHED is None:
        _CACHED = _build(ROWS_PER_CORE, N_CORES)
    return _CACHED


def run(inputs, trace=False, **kwargs):
    """Shard inputs over 8 cores, run SPMD, gather. Returns (out, BassKernelResults)."""
    dc0 = np.ascontiguousarray(inputs["dc0"], dtype=np.float32)
    dc1 = np.ascontiguousarray(inputs["dc1"], dtype=np.float32)
    dc2 = np.ascontiguousarray(inputs["dc2"], dtype=np.float32)
    assert dc0.shape == (BATCH, B0) and dc1.shape == (BATCH, B0 * B1)
    assert dc2.shape == (BATCH, B0 * B1 * B2)

    nc = _get_program()
    in_maps = []
    for c in range(N_CORES):
        sl = slice(c * ROWS_PER_CORE, (c + 1) * ROWS_PER_CORE)
        in_maps.append({"dc0": dc0[sl], "dc1": dc1[sl], "dc2": dc2[sl]})
    res = run_bass_kernel_spmd(
        nc, in_maps, core_ids=list(range(N_CORES)), trace=trace, **kwargs
    )
    out = np.concatenate([res.results[c]["out"] for c in range(N_CORES)], axis=0)
    return out.astype(np.float32), res


def kernel(**inputs) -> np.ndarray:
    out, _ = run(inputs, trace=False)
    return out
